# revision 28
# baseline (speedup 1.0000x reference)
"""Trainium2 Bass kernel for the OPU (optical matmul + ADC quantize) module.

Math (per r-block of 16 contraction rows, j = k mod 16):
    x_c = x + vmap_lut[j, x+8];  w_c = w + wmap_lut[j, w+8]
    out = sum_r round16(x_c[r] @ w_c[r])     (ADC clip never triggers)

Design:
  - K-sharded over 8 cores: core c owns contraction rows [128c, 128c+128)
    = 8 r-blocks, all BS=2048 tokens, all N=1024 columns. Quantized
    block results sum linearly, so the host adds the 8 fp16 partials.
  - fp8 (e4m3) DoubleRow matmuls at 0.5 cycles/row: operands split as
    exact integer part (xi, wi: 4-bit ints, fp8-exact) + small LUT
    correction (dx, dw). Per K=32 block, partition 2j holds xi_j,
    2j+1 holds dx_j (lhsT pair dim = stride-0 broadcast), and the rhs
    rows 2j/2j+1 both hold (A: wi_j, B: dw_j), giving
    xi*(wi+dw) + dx*(wi+dw) = x_c*w_c exactly.
  - ADC quantize inside PSUM: each chain is seeded with MAGIC = 1.5*2^27
    by a K=1 fp8e5 DR matmul (12288*16384); f32 ulp at MAGIC is 16, so
    every accumulated block rounds to a multiple of 16 (RNE), matching
    jnp.round. 32 chains = 16 token-tiles x 2 N-halves; each chain runs
    blocks at tile_position (0,0) then (32,0) (one position switch per
    accumulation group is the empirical HW limit; >=2 switches fail).
  - Operand assembly avoids partition-strided DMA writes (the tile
    framework loses deps on them): planes P=[xi|dx], Q=[wi|dw] are
    engine-written [128,*] tensors; operand tiles xt/xsh/wt/wsh are
    built by DMAs with contiguous-partition writes whose READ APs do
    the interleaving (pair-expansion / stride-0 row duplication).
  - Engine split: DVE runs only the custom LUT2ACC chains (2 levels per
    pass, 8 passes); Pool does casts/memsets; ACT evacuates PSUM->fp16
    with bias -MAGIC; ~33 DMA instructions total (each costs ~625ns of
    serialized HWDGE).
"""
import numpy as np
from contextlib import ExitStack

import concourse.bass as bass
import concourse.bacc as bacc
import concourse.tile as tile
import concourse.mybir as mybir
from concourse import bass_utils
from concourse import dve_ops
from concourse.dve_spec import Spec, Src0, Src1, C0, C1, C2, Zero, One, select, eq

F32 = mybir.dt.float32
FP16 = mybir.dt.float16
FP8 = mybir.dt.float8e4
FP8E5 = mybir.dt.float8e5

B, S, KDIM, N = 2, 1024, 1024, 1024
BS = B * S                        # 2048 tokens
NCORES = 8
TOK = BS                          # tokens per core (K-sharded: all)
NS = N                            # out cols per core (all)
NH = 512                          # N-half (PSUM bank free size)
MAGIC = float(3 * 2 ** 26)        # 1.5 * 2^27; f32 ulp there is exactly 16
SEED_A, SEED_B = 12288.0, 16384.0  # fp8e5 pair with product MAGIC

TOKBATCH = [256, 512, 512, 512, 256]   # token batch widths
TOKOFF = [0, 256, 768, 1280, 1792]

_cache = {}


def _register_lut2():
    name = "LUT2ACC"
    if name in dve_ops._SUB_OPCODE_FOR_NAME:
        return next(o for o in dve_ops.OPS if o.name == name)
    body = (
        Src0
        + select(eq(Src1, C2), C0, Zero)
        + select(eq(Src1, C2 + One), C1, Zero)
    )

    def ref(in0, in1, c0, c1, c2):
        r = in0.astype(np.float32) + \
            np.where(in1 == c2, c0, 0.0) + np.where(in1 == c2 + 1.0, c1, 0.0)
        return r.astype(np.float32)

    spec = Spec(body=body, reference=ref)
    op = dve_ops.DveOp(name, spec, subdim=False, uops_sha={})
    from concourse.dve_table_gen import dve_ver_for
    from concourse.dve_uop import DveOpSpec
    from concourse.dve_spec import lower, _has_src1
    ver = dve_ver_for("TRN2")
    opcode = max(dve_ops._SUB_OPCODE_FOR_NAME.values()) + 1
    assert opcode < 0x20
    dve_ops._SUB_OPCODE_FOR_NAME[name] = opcode
    lowered = DveOpSpec(name=name, opcode=opcode, uops=lower(spec, ver=ver),
                        rd1_en=_has_src1(spec))
    op.uops_sha[ver] = lowered.sha(ver)
    dve_ops.OPS.append(op)
    dve_ops.CUSTOM_DVE_SPECS[name] = spec
    return op


def _build(num_devices=NCORES):
    lut2 = _register_lut2()
    nc = bacc.Bacc("TRN2", target_bir_lowering=False, debug=False,
                   enable_asserts=False, num_devices=num_devices)
    x16_d = nc.dram_tensor("x16", [128, TOK], FP16, kind="ExternalInput").ap()
    w16_d = nc.dram_tensor("w16", [128, NS], FP16, kind="ExternalInput").ap()
    lut_d = nc.dram_tensor("luts", [128, 32], F32, kind="ExternalInput").ap()
    out_d = nc.dram_tensor("out", [BS, NS], FP16, kind="ExternalOutput").ap()

    with tile.TileContext(nc) as tc, ExitStack() as ctx:
        const = ctx.enter_context(tc.tile_pool(name="const", bufs=1))
        raw = ctx.enter_context(tc.tile_pool(name="raw", bufs=1))
        pp = ctx.enter_context(tc.tile_pool(name="pp", bufs=2))
        big = ctx.enter_context(tc.tile_pool(name="big", bufs=1))
        stg = ctx.enter_context(tc.tile_pool(name="stg", bufs=1))
        psum = ctx.enter_context(tc.tile_pool(name="psum", bufs=1, space="PSUM"))

        # ---------- constants / wires ----------
        luts = const.tile([128, 32], F32, tag="luts")
        nc.scalar.dma_start(luts[:], lut_d[:, :])
        zero16 = const.tile([128, 512], FP16, tag="zero16")
        nc.gpsimd.memset(zero16[:], 0.0)
        # e5m2 seed operands: pair A = (12288, 16384), pair B = (0, 0)
        sl = const.tile([1, 256], FP8E5, tag="sl")
        sr = const.tile([1, 2 * NH], FP8E5, tag="sr")
        nc.gpsimd.memset(sl[:], 0.0)
        nc.gpsimd.memset(sr[:], 0.0)
        nc.gpsimd.memset(sl[:, 0:128], SEED_A)
        nc.gpsimd.memset(sr[:, 0:NH], SEED_B)

        w16 = raw.tile([128, NS], FP16, tag="w16")
        nc.sync.dma_start(w16[:, 0:NH], w16_d[:, 0:NH])
        nc.sync.dma_start(w16[:, NH:], w16_d[:, NH:])
        x16 = raw.tile([128, TOK], FP16, tag="x16")
        for o, bw in zip(TOKOFF, TOKBATCH):
            nc.sync.dma_start(x16[:, o:o + bw], x16_d[:, o:o + bw])

        # ---------- planes (engine-written, contiguous partitions) ----------
        # P: [xi 0:2048 | dx 2048:4096], partition k = contraction row k
        P = big.tile([128, 2 * TOK], FP8, tag="P")
        # Q: [wi 0:1024 | dw 1024:2048]
        Q = big.tile([128, 2 * NS], FP8, tag="Q")
        # operand tiles: 64 partitions = rows (2j, 2j+1) for 32 j's
        # xt: blocks 0,1 (A cols) / 4,5 (B cols); xsh: blocks 2,3 / 6,7
        xt = big.tile([64, 2 * TOK], FP8, tag="xt")
        xsh = big.tile([64, 2 * TOK], FP8, tag="xsh")
        # wt cols: [A-wi 1024 | A-dw 1024 | B-wi 1024 | B-dw 1024]
        wt = big.tile([64, 4096], FP8, tag="wt")
        wsh = big.tile([64, 4096], FP8, tag="wsh")
        # base-0 copies of partitions 32:64 (chains keep tile_position (0,0))
        xt32 = big.tile([32, 2 * TOK], FP8, tag="xt32")
        xsh32 = big.tile([32, 2 * TOK], FP8, tag="xsh32")
        wt32 = big.tile([32, 4096], FP8, tag="wt32")
        wsh32 = big.tile([32, 4096], FP8, tag="wsh32")

        def lut_chain(dst, src, lutoff, col0, colw, tag):
            """dst = sum over levels of (src==lvl)*lut; 8 passes, fp8 out."""
            cur = zero16[:, 0:colw]
            for i in range(8):
                last = i == 7
                out = dst if last else pp.tile([128, colw], FP16,
                                               tag=f"pp{tag}{i % 2}",
                                               name=f"pp{tag}{i % 2}")
                outap = out if last else out[:]
                nc.vector._custom_dve(
                    lut2, out=outap, in0=cur, in1=src[:, col0:col0 + colw],
                    s0=luts[:, lutoff + 2 * i:lutoff + 2 * i + 1],
                    s1=luts[:, lutoff + 2 * i + 1:lutoff + 2 * i + 2],
                    imm2=float(2 * i - 8))
                if not last:
                    cur = out[:]

        def wi_asm():
            """wi columns of wt/wsh/wt32/wsh32: once, right after the cast
            (not latency-critical -> SWDGE)."""
            for dstt, p0, qn in ((wt, 0, "sync"), (wsh, 32, "scalar"),
                                 (wt, 64, "sync"), (wsh, 96, "scalar")):
                cg = 2 * (p0 // 64)
                src = (Q[p0:p0 + 32, 0:NS]
                       .unsqueeze(1).broadcast_to((32, 2, NS)))
                getattr(nc, qn).dma_start(dstt[:, 1024 * cg:1024 * cg + NS],
                                          src)
            for t64, t32, qn in ((wt, wt32, "sync"), (wsh, wsh32, "scalar")):
                getattr(nc, qn).dma_start(
                    t32[:, :].rearrange("p (g c) -> p g c", g=4)[:, 0::2, :],
                    t64[32:64, :].rearrange("p (g c) -> p g c", g=4)[:, 0::2, :])

        def wt_asm(nh):
            """dw columns of wt/wsh N-half nh (stride-0 row dup) + base-0
            shift copies; on the HWDGE queues (latency-critical)."""
            c0 = NH * nh
            for dstt, p0, qn in ((wt, 0, "sync"), (wsh, 32, "scalar"),
                                 (wt, 64, "sync"), (wsh, 96, "scalar")):
                cg = 2 * (p0 // 64)
                src = (Q[p0:p0 + 32, NS + c0:NS + c0 + NH]
                       .unsqueeze(1).broadcast_to((32, 2, NH)))
                dst = dstt[:, 1024 * (cg + 1) + c0:1024 * (cg + 1) + c0 + NH]
                getattr(nc, qn).dma_start(dst, src)
            for t64, t32, eng in ((wt, wt32, nc.sync), (wsh, wsh32, nc.scalar)):
                eng.dma_start(
                    t32[:, :].rearrange("p (g c) -> p g c", g=4)
                    [:, 1::2, c0:c0 + NH],
                    t64[32:64, :].rearrange("p (g c) -> p g c", g=4)
                    [:, 1::2, c0:c0 + NH])

        def xt_asm(bi):
            """Assemble xt/xsh token batch bi from P (pair expansion)."""
            o, bw = TOKOFF[bi], TOKBATCH[bi]
            XQ = {0: "sync", 32: "scalar", 64: "scalar", 96: "gpsimd"}
            for dstt, p0 in ((xt, 0), (xsh, 32), (xt, 64), (xsh, 96)):
                cg = 2048 * (p0 // 64)
                eng = getattr(nc, XQ[p0])
                src = (P[p0:p0 + 32, :]
                       .rearrange("j (two c) -> j two c", two=2)
                       [:, :, o:o + bw])
                eng.dma_start(dstt[:, cg + o:cg + o + bw], src)
            for t64, t32, eng in ((xt, xt32, nc.gpsimd), (xsh, xsh32, nc.sync)):
                eng.dma_start(
                    t32[:, :].rearrange("p (g c) -> p g c", g=2)[:, :, o:o + bw],
                    t64[32:64, :].rearrange("p (g c) -> p g c", g=2)
                    [:, :, o:o + bw])

        # ---------- fill planes + assemble (pipeline order) ----------
        # casts first (Pool), then W half 0, X batch 0, W half 1, X 1..3
        nc.gpsimd.tensor_scalar_add(Q[:, 0:NS], w16[:], 0.0)
        wi_asm()
        for o, bw in zip(TOKOFF, TOKBATCH):
            nc.gpsimd.tensor_scalar_add(P[:, o:o + bw], x16[:, o:o + bw], 0.0)

        def xchain(bi):
            o, bw = TOKOFF[bi], TOKBATCH[bi]
            lut_chain(P[:, TOK + o:TOK + o + bw], x16, 0, o, bw, f"x{bi}")
            xt_asm(bi)


        # ---------- chains ----------
        accs = [psum.tile([128, NH], F32, tag=f"acc{i}", name=f"acc{i}")
                for i in range(8)]
        stages = [stg.tile([128, 2048], FP16, tag=f"st{i}", name=f"st{i}")
                  for i in range(8)]

        # block b: g = b % 4, col group = b // 4; all operands at base 0,
        # every matmul at tile_position (0, 0)
        XT = {0: xt, 1: xt32, 2: xsh, 3: xsh32}
        WT = {0: wt, 1: wt32, 2: wsh, 3: wsh32}

        def lhs_ap(b, tt):
            g, cg = b % 4, b // 4
            colo = 2048 * cg + 128 * tt
            return (XT[g][0:32, colo:colo + 128]
                    .unsqueeze(1).broadcast_to((32, 2, 128)))

        def rhs_ap(b, nh):
            g, cg = b % 4, b // 4
            return (WT[g][0:32, :]
                    .rearrange("p (g c) -> p g c", g=4)
                    [:, 2 * cg:2 * cg + 2, NH * nh:NH * nh + NH])

        done = {}
        BLOCK_ORDER = [0, 2, 4, 6, 1, 3, 5, 7]  # shift-dependent blocks last
        chain_idx = [0]

        def chain(tt, nh):
            idx = chain_idx[0]
            chain_idx[0] += 1
            acc = accs[idx % 8]
            nc.tensor.matmul(acc[:],
                             sl[:].rearrange("p (two m) -> p two m", two=2),
                             sr[:].rearrange("p (two n) -> p two n", two=2),
                             start=True, stop=False,
                             perf_mode=mybir.MatmulPerfMode.DoubleRow,
                             tile_position=(0, 0))
            for bi, b in enumerate(BLOCK_ORDER):
                nc.tensor.matmul(acc[:], lhs_ap(b, tt), rhs_ap(b, nh),
                                 start=False, stop=bi == 7,
                                 perf_mode=mybir.MatmulPerfMode.DoubleRow,
                                 tile_position=(0, 0))
            # evac: fp16 partial (multiple of 16, |S| <= ~9k: exact)
            st = stages[tt // 2]
            co = 1024 * (tt % 2) + NH * nh
            nc.scalar.activation(st[:, co:co + NH], acc[:],
                                 mybir.ActivationFunctionType.Copy,
                                 bias=-MAGIC)
            if tt >= 14:
                nc.sync.dma_start(
                    out_d[128 * tt:128 * tt + 128, NH * nh:NH * nh + NH],
                    st[:, co:co + NH])
            else:
                done.setdefault(tt, set()).add(nh)
                tp = tt - tt % 2
                if done.get(tp) == {0, 1} and done.get(tp + 1) == {0, 1}:
                    nc.sync.dma_start(
                        out_d[128 * tp:128 * tp + 256, :]
                        .rearrange("(two r) n -> r two n", two=2),
                        st[:, :].rearrange("p (two n) -> p two n", two=2))

        # interleaved emission matched to DVE production order:
        # x0(256), w0, x1(512), x2(512), w1, x3(512), x4(256)
        # token-tile groups: [0-1], [2-5], [6-9], [10-13], [14-15]
        lut_chain(Q[:, NS:NS + NH], w16, 16, 0, NH, "w0")
        wt_asm(0)
        xchain(0)
        for tt in (0, 1):
            chain(tt, 0)
        xchain(1)
        for tt in (2, 3, 4, 5):
            chain(tt, 0)
        xchain(2)
        for tt in (6, 7, 8, 9):
            chain(tt, 0)
        lut_chain(Q[:, NS + NH:], w16, 16, NH, NH, "w1")
        wt_asm(1)
        for tt in range(10):
            chain(tt, 1)
        xchain(3)
        for tt in (10, 11, 12, 13):
            chain(tt, 0)
            chain(tt, 1)
        xchain(4)
        for tt in (14, 15):
            chain(tt, 0)
            chain(tt, 1)

    nc.compile()
    return nc


def kernel(input, weight, vmap_lut, wmap_lut):
    if "nc" not in _cache:
        _cache["nc"] = _build()
    nc = _cache["nc"]
    x2 = np.asarray(input, dtype=np.float32).reshape(BS, KDIM)
    w = np.asarray(weight, dtype=np.float32)
    vl = np.ascontiguousarray(np.asarray(vmap_lut, dtype=np.float32))
    wl = np.ascontiguousarray(np.asarray(wmap_lut, dtype=np.float32))
    luts = np.ascontiguousarray(
        np.concatenate([np.tile(vl, (8, 1)), np.tile(wl, (8, 1))], axis=1))
    in_maps = []
    for c in range(NCORES):
        k0 = 128 * c
        in_maps.append({
            "x16": np.ascontiguousarray(x2[:, k0:k0 + 128].T).astype(np.float16),
            "w16": w[k0:k0 + 128, :].astype(np.float16),
            "luts": luts,
        })
    res = bass_utils.run_bass_kernel_spmd(nc, in_maps,
                                          core_ids=list(range(NCORES)))
    out = np.zeros((BS, N), dtype=np.float32)
    for c in range(NCORES):
        out += res.results[c]["out"].astype(np.float32)
    return out.reshape(B, S, N)


# revision 36
# speedup vs baseline: 1.0445x; 1.0445x over previous
"""Trainium2 Bass kernel for the OPU (optical matmul + ADC quantize) module.

Math (per r-block of 16 contraction rows, j = k mod 16):
    x_c = x + vmap_lut[j, x+8];  w_c = w + wmap_lut[j, w+8]
    out = sum_r round16(x_c[r] @ w_c[r])     (ADC clip never triggers)

Design:
  - K-sharded over 8 cores: core c owns contraction rows [128c, 128c+128)
    = 8 r-blocks, all BS=2048 tokens, all N=1024 columns. Quantized
    block results sum linearly, so the host adds the 8 fp16 partials.
  - fp8 (e4m3) DoubleRow matmuls at 0.5 cycles/row: operands split as
    exact integer part (xi, wi: 4-bit ints, fp8-exact) + small LUT
    correction (dx, dw). Per K=32 block, partition 2j holds xi_j,
    2j+1 holds dx_j (lhsT pair dim = stride-0 broadcast), and the rhs
    rows 2j/2j+1 both hold (A: wi_j, B: dw_j), giving
    xi*(wi+dw) + dx*(wi+dw) = x_c*w_c exactly.
  - ADC quantize inside PSUM: each chain is seeded with MAGIC = 1.5*2^27
    by a K=1 fp8e5 DR matmul (12288*16384); f32 ulp at MAGIC is 16, so
    every accumulated block rounds to a multiple of 16 (RNE), matching
    jnp.round. 32 chains = 16 token-tiles x 2 N-halves; each chain runs
    blocks at tile_position (0,0) then (32,0) (one position switch per
    accumulation group is the empirical HW limit; >=2 switches fail).
  - Operand assembly avoids partition-strided DMA writes (the tile
    framework loses deps on them): planes P=[xi|dx], Q=[wi|dw] are
    engine-written [128,*] tensors; operand tiles xt/xsh/wt/wsh are
    built by DMAs with contiguous-partition writes whose READ APs do
    the interleaving (pair-expansion / stride-0 row duplication).
  - Engine split: DVE runs only the custom LUT2ACC chains (2 levels per
    pass, 8 passes); Pool does casts/memsets; ACT evacuates PSUM->fp16
    with bias -MAGIC; ~33 DMA instructions total (each costs ~625ns of
    serialized HWDGE).
"""
import numpy as np
from contextlib import ExitStack

import concourse.bass as bass
import concourse.bacc as bacc
import concourse.tile as tile
import concourse.mybir as mybir
from concourse import bass_utils
from concourse import dve_ops
from concourse.dve_spec import Spec, Src0, Src1, C0, C1, C2, Zero, One, select, eq

F32 = mybir.dt.float32
FP16 = mybir.dt.float16
FP8 = mybir.dt.float8e4
FP8E5 = mybir.dt.float8e5

B, S, KDIM, N = 2, 1024, 1024, 1024
BS = B * S                        # 2048 tokens
NCORES = 8
TOK = BS                          # tokens per core (K-sharded: all)
NS = N                            # out cols per core (all)
NH = 512                          # N-half (PSUM bank free size)
MAGIC = float(3 * 2 ** 26)        # 1.5 * 2^27; f32 ulp there is exactly 16
SEED_A, SEED_B = 12288.0, 16384.0  # fp8e5 pair with product MAGIC

TOKBATCH = [256, 512, 512, 512, 256]   # token batch widths
TOKOFF = [0, 256, 768, 1280, 1792]

_cache = {}


def _register_lut2():
    name = "LUT2ACC"
    if name in dve_ops._SUB_OPCODE_FOR_NAME:
        return next(o for o in dve_ops.OPS if o.name == name)
    body = (
        Src0
        + select(eq(Src1, C2), C0, Zero)
        + select(eq(Src1, C2 + One), C1, Zero)
    )

    def ref(in0, in1, c0, c1, c2):
        r = in0.astype(np.float32) + \
            np.where(in1 == c2, c0, 0.0) + np.where(in1 == c2 + 1.0, c1, 0.0)
        return r.astype(np.float32)

    spec = Spec(body=body, reference=ref)
    op = dve_ops.DveOp(name, spec, subdim=False, uops_sha={})
    from concourse.dve_table_gen import dve_ver_for
    from concourse.dve_uop import DveOpSpec
    from concourse.dve_spec import lower, _has_src1
    ver = dve_ver_for("TRN2")
    opcode = max(dve_ops._SUB_OPCODE_FOR_NAME.values()) + 1
    assert opcode < 0x20
    dve_ops._SUB_OPCODE_FOR_NAME[name] = opcode
    lowered = DveOpSpec(name=name, opcode=opcode, uops=lower(spec, ver=ver),
                        rd1_en=_has_src1(spec))
    op.uops_sha[ver] = lowered.sha(ver)
    dve_ops.OPS.append(op)
    dve_ops.CUSTOM_DVE_SPECS[name] = spec
    return op


def _build(num_devices=NCORES):
    lut2 = _register_lut2()
    nc = bacc.Bacc("TRN2", target_bir_lowering=False, debug=False,
                   enable_asserts=False, num_devices=num_devices)
    x16_d = nc.dram_tensor("x16", [128, TOK], FP16, kind="ExternalInput").ap()
    w16_d = nc.dram_tensor("w16", [128, NS], FP16, kind="ExternalInput").ap()
    lut_d = nc.dram_tensor("luts", [128, 32], F32, kind="ExternalInput").ap()
    out_d = nc.dram_tensor("out", [BS, NS], FP16, kind="ExternalOutput").ap()

    with tile.TileContext(nc) as tc, ExitStack() as ctx:
        const = ctx.enter_context(tc.tile_pool(name="const", bufs=1))
        raw = ctx.enter_context(tc.tile_pool(name="raw", bufs=1))
        pp = ctx.enter_context(tc.tile_pool(name="pp", bufs=2))
        big = ctx.enter_context(tc.tile_pool(name="big", bufs=1))
        stg = ctx.enter_context(tc.tile_pool(name="stg", bufs=1))
        psum = ctx.enter_context(tc.tile_pool(name="psum", bufs=1, space="PSUM"))

        # ---------- constants / wires ----------
        luts = const.tile([128, 32], F32, tag="luts")
        nc.scalar.dma_start(luts[:], lut_d[:, :])
        zero16 = const.tile([128, 512], FP16, tag="zero16")
        nc.gpsimd.memset(zero16[:], 0.0)
        # e5m2 seed operands: pair A = (12288, 16384), pair B = (0, 0)
        sl = const.tile([1, 256], FP8E5, tag="sl")
        sr = const.tile([1, 2 * NH], FP8E5, tag="sr")
        nc.gpsimd.memset(sl[:], 0.0)
        nc.gpsimd.memset(sr[:], 0.0)
        nc.gpsimd.memset(sl[:, 0:128], SEED_A)
        nc.gpsimd.memset(sr[:, 0:NH], SEED_B)

        w16 = raw.tile([128, NS], FP16, tag="w16")
        nc.sync.dma_start(w16[:, 0:NH], w16_d[:, 0:NH])
        nc.sync.dma_start(w16[:, NH:], w16_d[:, NH:])
        x16 = raw.tile([128, TOK], FP16, tag="x16")
        for o, bw in zip(TOKOFF, TOKBATCH):
            nc.sync.dma_start(x16[:, o:o + bw], x16_d[:, o:o + bw])

        # ---------- planes (engine-written, contiguous partitions) ----------
        # P: [xi 0:2048 | dx 2048:4096], partition k = contraction row k
        P = big.tile([128, 2 * TOK], FP8, tag="P")
        # Q: [wi 0:1024 | dw 1024:2048]
        Q = big.tile([128, 2 * NS], FP8, tag="Q")
        # operand tiles: 64 partitions = rows (2j, 2j+1) for 32 j's
        # xt: blocks 0,1 (A cols) / 4,5 (B cols); xsh: blocks 2,3 / 6,7
        xt = big.tile([64, 2 * TOK], FP8, tag="xt")
        xsh = big.tile([64, 2 * TOK], FP8, tag="xsh")
        # wt cols: [A-wi 1024 | A-dw 1024 | B-wi 1024 | B-dw 1024]
        wt = big.tile([64, 4096], FP8, tag="wt")
        wsh = big.tile([64, 4096], FP8, tag="wsh")
        # base-0 copies of partitions 32:64 (chains keep tile_position (0,0))
        xt32 = big.tile([32, 2 * TOK], FP8, tag="xt32")
        xsh32 = big.tile([32, 2 * TOK], FP8, tag="xsh32")
        wt32 = big.tile([32, 4096], FP8, tag="wt32")
        wsh32 = big.tile([32, 4096], FP8, tag="wsh32")

        def lut_chain(dst, src, lutoff, col0, colw, tag):
            """dst = sum over levels of (src==lvl)*lut; 8 passes, fp8 out."""
            cur = zero16[:, 0:colw]
            for i in range(8):
                last = i == 7
                out = dst if last else pp.tile([128, colw], FP16,
                                               tag=f"pp{tag}{i % 2}",
                                               name=f"pp{tag}{i % 2}")
                outap = out if last else out[:]
                nc.vector._custom_dve(
                    lut2, out=outap, in0=cur, in1=src[:, col0:col0 + colw],
                    s0=luts[:, lutoff + 2 * i:lutoff + 2 * i + 1],
                    s1=luts[:, lutoff + 2 * i + 1:lutoff + 2 * i + 2],
                    imm2=float(2 * i - 8))
                if not last:
                    cur = out[:]

        def wi_asm():
            """wi columns of wt/wsh/wt32/wsh32: once, right after the cast
            (not latency-critical -> SWDGE)."""
            for dstt, p0, qn in ((wt, 0, "sync"), (wsh, 32, "scalar"),
                                 (wt, 64, "sync"), (wsh, 96, "scalar")):
                cg = 2 * (p0 // 64)
                src = (Q[p0:p0 + 32, 0:NS]
                       .unsqueeze(1).broadcast_to((32, 2, NS)))
                getattr(nc, qn).dma_start(dstt[:, 1024 * cg:1024 * cg + NS],
                                          src)
            for t64, t32, qn in ((wt, wt32, "sync"), (wsh, wsh32, "scalar")):
                getattr(nc, qn).dma_start(
                    t32[:, :].rearrange("p (g c) -> p g c", g=4)[:, 0::2, :],
                    t64[32:64, :].rearrange("p (g c) -> p g c", g=4)[:, 0::2, :])

        def wt_asm(nh):
            """dw columns of wt/wsh N-half nh (stride-0 row dup) + base-0
            shift copies; on the HWDGE queues (latency-critical)."""
            c0 = NH * nh
            for dstt, p0, qn in ((wt, 0, "sync"), (wsh, 32, "scalar"),
                                 (wt, 64, "gpsimd"), (wsh, 96, "gpsimd")):
                cg = 2 * (p0 // 64)
                src = (Q[p0:p0 + 32, NS + c0:NS + c0 + NH]
                       .unsqueeze(1).broadcast_to((32, 2, NH)))
                dst = dstt[:, 1024 * (cg + 1) + c0:1024 * (cg + 1) + c0 + NH]
                getattr(nc, qn).dma_start(dst, src)
            for t64, t32, eng in ((wt, wt32, nc.scalar), (wsh, wsh32, nc.sync)):
                eng.dma_start(
                    t32[:, :].rearrange("p (g c) -> p g c", g=4)
                    [:, 1::2, c0:c0 + NH],
                    t64[32:64, :].rearrange("p (g c) -> p g c", g=4)
                    [:, 1::2, c0:c0 + NH])

        def xt_asm(bi):
            """Assemble xt/xsh token batch bi from P (pair expansion)."""
            o, bw = TOKOFF[bi], TOKBATCH[bi]
            XQ = {0: "sync", 32: "scalar", 64: "scalar", 96: "gpsimd"}
            for dstt, p0 in ((xt, 0), (xsh, 32), (xt, 64), (xsh, 96)):
                cg = 2048 * (p0 // 64)
                eng = getattr(nc, XQ[p0])
                src = (P[p0:p0 + 32, :]
                       .rearrange("j (two c) -> j two c", two=2)
                       [:, :, o:o + bw])
                eng.dma_start(dstt[:, cg + o:cg + o + bw], src)
            for t64, t32, eng in ((xt, xt32, nc.sync), (xsh, xsh32, nc.scalar)):
                eng.dma_start(
                    t32[:, :].rearrange("p (g c) -> p g c", g=2)[:, :, o:o + bw],
                    t64[32:64, :].rearrange("p (g c) -> p g c", g=2)
                    [:, :, o:o + bw])

        # ---------- fill planes + assemble (pipeline order) ----------
        # casts first (Pool), then W half 0, X batch 0, W half 1, X 1..3
        nc.gpsimd.tensor_scalar_add(Q[:, 0:NS], w16[:], 0.0)
        wi_asm()
        for o, bw in zip(TOKOFF, TOKBATCH):
            nc.gpsimd.tensor_scalar_add(P[:, o:o + bw], x16[:, o:o + bw], 0.0)

        def xchain(bi):
            o, bw = TOKOFF[bi], TOKBATCH[bi]
            lut_chain(P[:, TOK + o:TOK + o + bw], x16, 0, o, bw, f"x{bi}")
            xt_asm(bi)


        # ---------- chains ----------
        accs = [psum.tile([128, NH], F32, tag=f"acc{i}", name=f"acc{i}")
                for i in range(8)]
        stages = [stg.tile([128, 2048], FP16, tag=f"st{i}", name=f"st{i}")
                  for i in range(8)]

        # block b: g = b % 4, col group = b // 4; all operands at base 0,
        # every matmul at tile_position (0, 0)
        XT = {0: xt, 1: xt32, 2: xsh, 3: xsh32}
        WT = {0: wt, 1: wt32, 2: wsh, 3: wsh32}

        def lhs_ap(b, tt):
            g, cg = b % 4, b // 4
            colo = 2048 * cg + 128 * tt
            return (XT[g][0:32, colo:colo + 128]
                    .unsqueeze(1).broadcast_to((32, 2, 128)))

        def rhs_ap(b, nh):
            g, cg = b % 4, b // 4
            return (WT[g][0:32, :]
                    .rearrange("p (g c) -> p g c", g=4)
                    [:, 2 * cg:2 * cg + 2, NH * nh:NH * nh + NH])

        done = {}
        BLOCK_ORDER = [0, 2, 4, 6, 1, 3, 5, 7]  # shift-dependent blocks last
        chain_idx = [0]

        def chain(tt, nh):
            idx = chain_idx[0]
            chain_idx[0] += 1
            acc = accs[idx % 8]
            nc.tensor.matmul(acc[:],
                             sl[:].rearrange("p (two m) -> p two m", two=2),
                             sr[:].rearrange("p (two n) -> p two n", two=2),
                             start=True, stop=False,
                             perf_mode=mybir.MatmulPerfMode.DoubleRow,
                             tile_position=(0, 0))
            for bi, b in enumerate(BLOCK_ORDER):
                nc.tensor.matmul(acc[:], lhs_ap(b, tt), rhs_ap(b, nh),
                                 start=False, stop=bi == 7,
                                 perf_mode=mybir.MatmulPerfMode.DoubleRow,
                                 tile_position=(0, 0))
            # evac: fp16 partial (multiple of 16, |S| <= ~9k: exact)
            st = stages[tt // 2]
            co = 1024 * (tt % 2) + NH * nh
            nc.scalar.activation(st[:, co:co + NH], acc[:],
                                 mybir.ActivationFunctionType.Copy,
                                 bias=-MAGIC)
            if tt >= 12:
                nc.sync.dma_start(
                    out_d[128 * tt:128 * tt + 128, NH * nh:NH * nh + NH],
                    st[:, co:co + NH])
            else:
                done.setdefault(tt, set()).add(nh)
                if done[tt] == {0, 1}:
                    nc.sync.dma_start(
                        out_d[128 * tt:128 * tt + 128, :],
                        st[:, 1024 * (tt % 2):1024 * (tt % 2) + 1024])

        # interleaved emission matched to DVE production order:
        # x0(256), w0, x1(512), x2(512), w1, x3(512), x4(256)
        # token-tile groups: [0-1], [2-5], [6-9], [10-13], [14-15]
        lut_chain(Q[:, NS:NS + NH], w16, 16, 0, NH, "w0")
        wt_asm(0)
        xchain(0)
        for tt in (0, 1):
            chain(tt, 0)
        xchain(1)
        for tt in (2, 3, 4, 5):
            chain(tt, 0)
        xchain(2)
        for tt in (6, 7, 8, 9):
            chain(tt, 0)
        lut_chain(Q[:, NS + NH:], w16, 16, NH, NH, "w1")
        wt_asm(1)
        for tt in range(10):
            chain(tt, 1)
        xchain(3)
        for tt in (10, 11, 12, 13):
            chain(tt, 0)
            chain(tt, 1)
        xchain(4)
        for tt in (14, 15):
            chain(tt, 0)
            chain(tt, 1)

    nc.compile()
    return nc


def kernel(input, weight, vmap_lut, wmap_lut):
    if "nc" not in _cache:
        _cache["nc"] = _build()
    nc = _cache["nc"]
    x2 = np.asarray(input, dtype=np.float32).reshape(BS, KDIM)
    w = np.asarray(weight, dtype=np.float32)
    vl = np.ascontiguousarray(np.asarray(vmap_lut, dtype=np.float32))
    wl = np.ascontiguousarray(np.asarray(wmap_lut, dtype=np.float32))
    luts = np.ascontiguousarray(
        np.concatenate([np.tile(vl, (8, 1)), np.tile(wl, (8, 1))], axis=1))
    in_maps = []
    for c in range(NCORES):
        k0 = 128 * c
        in_maps.append({
            "x16": np.ascontiguousarray(x2[:, k0:k0 + 128].T).astype(np.float16),
            "w16": w[k0:k0 + 128, :].astype(np.float16),
            "luts": luts,
        })
    res = bass_utils.run_bass_kernel_spmd(nc, in_maps,
                                          core_ids=list(range(NCORES)))
    out = np.zeros((BS, N), dtype=np.float32)
    for c in range(NCORES):
        out += res.results[c]["out"].astype(np.float32)
    return out.reshape(B, S, N)
